# revision 11
# baseline (speedup 1.0000x reference)
"""Causal self-attention (B=4, T=2048, C=1024, H=16) on 8 trn2 NeuronCores.

Sharding: core c = (batch b = c//2, head-group g = c%2). Each core computes
the full attention for batch b and heads 8g..8g+7 (column-parallel qkv,
row-parallel proj), producing a partial [T, C] output; the host sums the two
partials per batch.

Per-core device kernel (Bass/Tile, SPMD same program on all 8 cores):
  qT/kT  [512, T] = (wq|wk).T @ x.T        (bf16 matmuls, fp32 psum)
  v      [T, 8, 65]  (natural layout, ones column appended per head)
  S^T    [tk 128, tq 512] blocks = kT.T-slices @ qT-slices (2 heads
         row-packed; the K=64 pair runs concurrently via PE row tiling)
  P^T    = exp((S^T + causal_mask)/8)      (ScalarE, psum->sbuf bf16)
  y/l    = [v|1].T @ P^T  accumulated over tk  -> [65, tq] psum per head
  yT_n   = yT * replicate(1/l)             (recip on DVE; the two replicate
         matmuls run concurrently as PE column tiles (64,0)/(64,64))
  out    = yT_n.T @ wo  -> [T, C] fp32 partial

Scheduling: DMA order wv,x,wq,wk,wo; v tiles 0-3 contract k-major so they
consume x chunks as they stream in; v tiles / qk(0,j) / attn(0,j) are
braided, later head-pairs braid qk(m,j)+attn(m,j), and proj is emitted
per-j inside the last attention stage so the PE never starves (HAM warm).
"""

import os
import sys

import numpy as np

import concourse.bacc as bacc
import concourse.bass as bass
import concourse.mybir as mybir
import concourse.tile as tile
from concourse.bass_utils import run_bass_kernel_spmd

try:
    import ml_dtypes

    BF16 = np.dtype(ml_dtypes.bfloat16)
except ImportError:  # pragma: no cover
    BF16 = np.dtype("bfloat16")

B, T, C = 4, 2048, 1024
N_HEAD = 16
D = 64  # head dim
H_LOC = 8  # heads per core
DL = H_LOC * D  # 512, local d width per core
CK = C // 128  # 8 contraction chunks
DT = mybir.dt.bfloat16
F32 = mybir.dt.float32
NEG = -1.0e9


def build_program(t_len=T, enable_asserts=False):
    """Build the SPMD per-core program. Returns the compiled Bacc object."""
    NJ = t_len // 512  # tq chunks
    NTT = t_len // 128  # 128-wide t tiles
    MD = DL // 128  # 4 d-chunks of qT/kT/yT

    nc = bacc.Bacc(
        "TRN2",
        target_bir_lowering=False,
        debug=False,
        enable_asserts=enable_asserts,
        num_devices=8,
    )

    xT_d = nc.dram_tensor("xT", [C, t_len], DT, kind="ExternalInput").ap()
    wq_d = nc.dram_tensor("wq", [C, DL], DT, kind="ExternalInput").ap()
    wk_d = nc.dram_tensor("wk", [C, DL], DT, kind="ExternalInput").ap()
    wv_d = nc.dram_tensor("wv", [C, DL], DT, kind="ExternalInput").ap()
    wo_d = nc.dram_tensor("wo", [DL, C], DT, kind="ExternalInput").ap()
    mask_d = nc.dram_tensor("mask", [128, 128], F32, kind="ExternalInput").ap()
    out_d = nc.dram_tensor("out", [t_len, C], F32, kind="ExternalOutput").ap()

    with tile.TileContext(nc) as tc:
        with (
            tc.tile_pool(name="consts", bufs=1) as cpool,
            tc.tile_pool(name="ptp", bufs=6) as pt_pool,
            tc.tile_pool(name="yup", bufs=3) as yu_pool,
            tc.tile_pool(name="rlp", bufs=3) as rl_pool,
            tc.tile_pool(name="outp", bufs=3) as out_pool,
            tc.tile_pool(name="psum", bufs=1, space="PSUM") as psum,
        ):
            # ---- persistent SBUF tensors ----
            xt_t = cpool.tile([128, CK, t_len], DT, name="xt")
            wq_t = cpool.tile([128, CK, DL], DT, name="wqt")
            wk_t = cpool.tile([128, CK, DL], DT, name="wkt")
            wv_t = cpool.tile([128, CK, DL], DT, name="wvt")
            wo_t = cpool.tile([128, MD, C], DT, name="wot")
            qt_t = cpool.tile([128, MD, t_len], DT, name="qtt")
            kt_t = cpool.tile([128, MD, t_len], DT, name="ktt")
            v_t = cpool.tile([128, NTT, H_LOC, D + 1], DT, name="vt")
            yt_t = cpool.tile([128, MD, t_len], DT, name="ytt")
            mask_t = cpool.tile([128, 2, 128], F32, name="maskt")
            ones1_t = cpool.tile([D + 1, 64], DT, name="ones1t")

            # ---- input DMAs: few big transfers (per-dma issue cost on the
            # Sync engine is ~0.6us); wv + x first (v-stage inputs), wo last
            xT_v = xT_d.rearrange("(k p) t -> p k t", p=128)
            wq_v = wq_d.rearrange("(k p) d -> p k d", p=128)
            wk_v = wk_d.rearrange("(k p) d -> p k d", p=128)
            wv_v = wv_d.rearrange("(k p) d -> p k d", p=128)
            wo_v = wo_d.rearrange("(m p) c -> p m c", p=128)
            nc.sync.dma_start(out=mask_t[:, 0, :], in_=mask_d)
            nc.sync.dma_start(out=mask_t[:, 1, :], in_=mask_d)
            nc.sync.dma_start(out=wv_t[:, :, :], in_=wv_v)
            for k in range(CK):
                nc.sync.dma_start(out=xt_t[:, k, :], in_=xT_v[:, k, :])
            # wq/wk/wo DMAs are emitted after v_wave0 below so their issue
            # (and transfer start) trails the x chunks on the Sync engine
            # ones column (index 64) for the l (softmax denominator) rows
            nc.vector.memset(v_t[:, :, :, D : D + 1], 1.0)
            nc.vector.memset(ones1_t[:, :], 1.0)

            # ---- qkv ----
            def qk_proj(w_t, dst_t, m, j):
                ps = psum.tile([128, 512], F32, name="qkvps", bufs=2)
                for k in range(CK):
                    nc.tensor.matmul(
                        ps[:, :],
                        lhsT=w_t[:, k, 128 * m : 128 * (m + 1)],
                        rhs=xt_t[:, k, 512 * j : 512 * (j + 1)],
                        start=(k == 0),
                        stop=(k == CK - 1),
                    )
                nc.vector.tensor_copy(dst_t[:, m, 512 * j : 512 * (j + 1)], ps[:, :])

            def v_wave0(tis):
                # k-major: consume the two x half-transfers as they arrive
                spsT = psum.tile([128, 2, 512], F32, name="sps", bufs=2)
                vps = [
                    psum.tile([128, 512], F32, name="qkvps", bufs=2),
                    psum.tile([128, 512], F32, name="qkvps", bufs=2),
                    spsT[:, 0, :],
                    spsT[:, 1, :],
                ]
                for k in range(CK):
                    for idx, ti in enumerate(tis):
                        nc.tensor.matmul(
                            vps[idx],
                            lhsT=xt_t[:, k, 128 * ti : 128 * (ti + 1)],
                            rhs=wv_t[:, k, :],
                            start=(k == 0),
                            stop=(k == CK - 1),
                        )
                for idx, ti in enumerate(tis):
                    nc.vector.tensor_copy(
                        v_t[:, ti, :, 0:D],
                        vps[idx].rearrange("p (h d) -> p h d", h=H_LOC),
                    )

            def v_tile(ti):
                ps = psum.tile([128, 512], F32, name="qkvps", bufs=2)
                for k in range(CK):
                    nc.tensor.matmul(
                        ps[:, :],
                        lhsT=xt_t[:, k, 128 * ti : 128 * (ti + 1)],
                        rhs=wv_t[:, k, :],
                        start=(k == 0),
                        stop=(k == CK - 1),
                    )
                nc.vector.tensor_copy(
                    v_t[:, ti, :, 0:D],
                    ps[:, :].rearrange("p (h d) -> p h d", h=H_LOC),
                )

            def attn_j(hp, j):
                tq0 = 512 * j
                nblk = 4 * j + 4  # causal: tk blocks 0 .. 4j+3
                # accA/accB live in one 2-bank tile; after the yu copy the
                # dead banks are reused for the replicate-l matmul outputs so
                # the normalization chain never holds an sps/qkvps slot.
                acc = psum.tile([128, 2, 512], F32, name="acc", bufs=1)
                pend = []  # software pipeline: AV for block i-1 after S of i

                def flush_av():
                    for mm in pend:
                        nc.tensor.matmul(**mm)
                    pend.clear()

                for i in range(nblk):
                    tk = slice(128 * i, 128 * (i + 1))
                    diag = i - 4 * j
                    lo = 128 * diag if diag >= 0 else 0
                    tqs = slice(tq0 + lo, tq0 + 512)
                    sps = psum.tile([128, 2, 512], F32, name="sps", bufs=2)
                    for h2, lohi in ((0, slice(0, 64)), (1, slice(64, 128))):
                        nc.tensor.matmul(
                            sps[:, h2, lo:],
                            lhsT=kt_t[lohi, hp, tk],
                            rhs=qt_t[lohi, hp, tqs],
                            start=True,
                            stop=True,
                        )
                    if diag >= 0:  # block crosses the causal diagonal
                        dg = slice(lo, lo + 128)
                        nc.vector.tensor_add(
                            sps[:, :, dg], sps[:, :, dg], mask_t[:, :, :]
                        )
                    pt = pt_pool.tile([128, 2, 512], DT, name="pt")
                    nc.scalar.activation(
                        pt[:, :, lo:],
                        sps[:, :, lo:],
                        mybir.ActivationFunctionType.Exp,
                        scale=0.125,
                    )
                    flush_av()
                    for h2 in range(2):
                        pend.append(
                            dict(
                                out=acc[0 : D + 1, h2, lo:],
                                lhsT=v_t[:, i, 2 * hp + h2, :],
                                rhs=pt[:, h2, lo:],
                                start=(i == 0),
                                stop=(i == nblk - 1),
                            )
                        )
                flush_av()

                # normalization: replicate l via K=1 matmuls into the dead
                # acc banks, recip, multiply
                tq = slice(tq0, tq0 + 512)
                yuA = yu_pool.tile([D + 1, 512], DT, name="yuA")
                yuB = yu_pool.tile([D + 1, 512], DT, name="yuB")
                nc.vector.tensor_copy(yuA[:, :], acc[0 : D + 1, 0, :])
                nc.vector.tensor_copy(yuB[:, :], acc[0 : D + 1, 1, :])
                nc.tensor.matmul(
                    acc[0:64, 0, :],
                    lhsT=ones1_t[D : D + 1, :],
                    rhs=yuA[D : D + 1, :],
                    start=True,
                    stop=True,
                    tile_position=(64, 0),
                )
                nc.tensor.matmul(
                    acc[0:64, 1, :],
                    lhsT=ones1_t[D : D + 1, :],
                    rhs=yuB[D : D + 1, :],
                    start=True,
                    stop=True,
                    tile_position=(64, 0),
                )
                rliA = rl_pool.tile([64, 512], F32, name="rliA")
                rliB = rl_pool.tile([64, 512], F32, name="rliB")
                nc.vector.reciprocal_approx_fast(rliA[:, :], acc[0:64, 0, :])
                nc.vector.reciprocal_approx_fast(rliB[:, :], acc[0:64, 1, :])
                nc.vector.tensor_mul(yt_t[0:64, hp, tq], yuA[0:D, :], rliA[:, :])
                nc.vector.tensor_mul(yt_t[64:128, hp, tq], yuB[0:D, :], rliB[:, :])

            def proj_tile(ti):
                tt = slice(128 * ti, 128 * (ti + 1))
                ot = out_pool.tile([128, C], F32, name="ot")
                for ci in range(2):
                    cs = slice(512 * ci, 512 * (ci + 1))
                    ps = psum.tile([128, 512], F32, name="qkvps", bufs=2)
                    for hp2 in range(MD):
                        nc.tensor.matmul(
                            ps[:, :],
                            lhsT=yt_t[:, hp2, tt],
                            rhs=wo_t[:, hp2, cs],
                            start=(hp2 == 0),
                            stop=(hp2 == MD - 1),
                        )
                    nc.vector.tensor_copy(ot[:, cs], ps[:, :])
                nc.sync.dma_start(out=out_d[tt, :], in_=ot[:, :])

            # ---- braided schedule ----
            v_wave0([0, 1, 2, 3])
            nc.sync.dma_start(out=wq_t[:, :, :], in_=wq_v)
            nc.sync.dma_start(out=wk_t[:, :, :], in_=wk_v)
            nc.sync.dma_start(out=wo_t[:, :, :], in_=wo_v)
            for j in range(NJ):
                qk_proj(wq_t, qt_t, 0, j)
                qk_proj(wk_t, kt_t, 0, j)
                attn_j(0, j)
                if j < NJ - 1:
                    for ti in range(4 * (j + 1), 4 * (j + 2)):
                        v_tile(ti)
            for m in range(1, MD - 1):
                for j in range(NJ):
                    qk_proj(wq_t, qt_t, m, j)
                    qk_proj(wk_t, kt_t, m, j)
                    attn_j(m, j)
            # last stage: qk first, then attention j-descending so the kernel
            # ends on the smallest chunk, proj braided in as PE filler
            m = MD - 1
            for j in range(NJ):
                qk_proj(wq_t, qt_t, m, j)
                qk_proj(wk_t, kt_t, m, j)
            for j in range(NJ - 1, -1, -1):
                attn_j(m, j)
                for ti in range(4 * j, 4 * (j + 1)):
                    proj_tile(ti)

    nc.compile()
    return nc


def make_host_inputs(x, w_qkv, w_proj, t_len=T):
    """Shard full inputs into the 8 per-core input dicts."""
    mask = np.where(
        np.arange(128)[None, :] >= np.arange(128)[:, None], 0.0, NEG
    ).astype(np.float32)

    in_maps = []
    for c in range(8):
        b, g = c // 2, c % 2
        xT = np.ascontiguousarray(x[b][:t_len].T).astype(BF16)
        wq = w_qkv[:, 512 * g : 512 * (g + 1)].astype(BF16)
        wk = w_qkv[:, C + 512 * g : C + 512 * (g + 1)].astype(BF16)
        wv = w_qkv[:, 2 * C + 512 * g : 2 * C + 512 * (g + 1)].astype(BF16)
        wo = np.ascontiguousarray(w_proj[512 * g : 512 * (g + 1), :]).astype(BF16)
        in_maps.append(dict(xT=xT, wq=wq, wk=wk, wv=wv, wo=wo, mask=mask))
    return in_maps


_CACHE = {}


def _get_program():
    if "nc" not in _CACHE:
        _CACHE["nc"] = build_program()
    return _CACHE["nc"]


def kernel(x, w_qkv, w_proj, _trace=False, _trace_kwargs=None):
    x = np.asarray(x, np.float32)
    w_qkv = np.asarray(w_qkv, np.float32)
    w_proj = np.asarray(w_proj, np.float32)
    nc = _get_program()
    in_maps = make_host_inputs(x, w_qkv, w_proj)
    kw = {}
    if _trace:
        kw = dict(trace=True, **(_trace_kwargs or {}))
    res = run_bass_kernel_spmd(nc, in_maps, core_ids=list(range(8)), **kw)
    out = np.empty((B, T, C), np.float32)
    for b in range(B):
        out[b] = res.results[2 * b]["out"] + res.results[2 * b + 1]["out"]
    if _trace:
        return out, res
    return out
